# revision 1
# baseline (speedup 1.0000x reference)
"""AdaptiveStdPooling2d on 8 TRN2 NeuronCores.

Input  x: [16, 128, 512, 128] f32.  Output: [16, 128, 8, 16] f32.
out[b,c,i,j] = sum_{kw=0..7} std_h(x[b, c, 64*i:64*i+64, 8*j+kw])
with biased variance over the 64-row bin plus EPS=1e-14 inside sqrt.

Sharding: pure data parallel over batch B=16 -> 2 per core, no collectives.

Per-core kernel: C=128 channels on SBUF partitions.  For each (b, bin_h)
DMA a [128c, 64h, 128w] tile (contiguous 32 KiB per partition), square it
on the scalar engine, segmented reduce_sum over the 64-row bins on the
vector engine for both x and x^2, var = E[x^2]-E[x]^2, sqrt on the scalar
engine, reduce_sum over kw=8 into the output tile.
"""

import numpy as np

B, C, H, W = 16, 128, 512, 128
N_CORES = 8
B_LOC = B // N_CORES          # 2 batches per core
H_OUT, W_OUT = 8, 16
KH, KW = H // H_OUT, W // W_OUT   # 64, 8
EPS = 1e-14

_CACHE = {}


def _build_bass(reps=1, variant="full"):
    import concourse.bacc as bacc
    import concourse.mybir as mybir
    from concourse import tile

    f32 = mybir.dt.float32
    nc = bacc.Bacc(None, target_bir_lowering=False)
    x_in = nc.declare_dram_parameter("x", [B_LOC, C, H, W], f32, isOutput=False)
    out = nc.declare_dram_parameter("out", [B_LOC, C, H_OUT, W_OUT], f32, isOutput=True)

    with tile.TileContext(nc) as tc:
        with (
            tc.tile_pool(name="xp", bufs=3) as xp,
            tc.tile_pool(name="sq", bufs=2) as sqp,
            tc.tile_pool(name="tp", bufs=3) as tp,
            tc.tile_pool(name="op", bufs=1) as op,
        ):
            oacc = op.tile([C, B_LOC, H_OUT, W_OUT], f32, tag="oacc")
            eps_t = op.tile([C, 1], f32, tag="eps")
            nc.vector.memset(eps_t[:], float(EPS))
            if variant == "dma":
                nc.vector.memset(oacc[:], 0.0)
            for b, ih in [(b, ih) for _ in range(reps)
                          for b in range(B_LOC) for ih in range(H_OUT)]:
                    xt = xp.tile([C, KH, W], f32, tag="x")
                    nc.sync.dma_start(out=xt[:], in_=x_in[b, :, ih * KH:(ih + 1) * KH, :])
                    if variant == "dma":
                        continue
                    s1 = tp.tile([C, W], f32, tag="s1")
                    s2 = tp.tile([C, W], f32, tag="s2")
                    if variant in ("full", "nosq"):
                        if variant == "full":
                            sq = sqp.tile([C, KH, W], f32, tag="sq")
                            nc.scalar.activation(
                                sq[:], xt[:], mybir.ActivationFunctionType.Square,
                            )
                        else:
                            sq = xt
                        nc.vector.reduce_sum(
                            out=s2[:], in_=sq[:].transpose([0, 2, 1]),
                            axis=mybir.AxisListType.X,
                        )
                    # reduce over the h axis (innermost after transpose)
                    nc.vector.reduce_sum(
                        out=s1[:], in_=xt[:].transpose([0, 2, 1]),
                        axis=mybir.AxisListType.X,
                    )
                    if variant == "onepass":
                        s2 = s1
                    # var = s2/64 - (s1/64)^2
                    mean = tp.tile([C, W], f32, tag="mean")
                    m2 = tp.tile([C, W], f32, tag="m2")
                    var = tp.tile([C, W], f32, tag="var")
                    stds = tp.tile([C, W], f32, tag="stds")
                    nc.vector.tensor_scalar_mul(mean[:], s1[:], 1.0 / KH)
                    nc.vector.tensor_mul(m2[:], mean[:], mean[:])
                    nc.vector.scalar_tensor_tensor(
                        out=var[:], in0=s2[:], scalar=1.0 / KH, in1=m2[:],
                        op0=mybir.AluOpType.mult, op1=mybir.AluOpType.subtract,
                    )
                    nc.scalar.activation(
                        stds[:], var[:], mybir.ActivationFunctionType.Sqrt,
                        bias=eps_t[:], scale=1.0,
                    )
                    nc.vector.reduce_sum(
                        out=oacc[:, b, ih, :],
                        in_=stds[:].rearrange("p (g k) -> p g k", k=KW),
                        axis=mybir.AxisListType.X,
                    )
            nc.sync.dma_start(out=out.transpose([1, 0, 2, 3]), in_=oacc[:])
    nc.finalize()
    return nc


def _build_pe(reps=1, variant="full", n_dve=0, sq_alt=False, gp_dve=False,
              single_packet=False, loop_reps=1):
    """Pure TensorEngine reduction path.

    Layout: h (within a 128-row chunk = 2 bins) on partitions.  Per chunk
    (b, hc, cc) of [128h, 64c, 128w]:
      DVE casts x -> bf16, ACT squares x -> bf16, PE reduces both over the
      two 64-row bins via ldweights(X)/matmul(selector) pairs (out [128w, 2]
      per channel), DVE combines into var (fp32), ACT sqrt, and a second
      tiny PE matmul against a kw-selector sums std over w-groups of 8,
      landing [c, w_out] with c back on partitions.
    """
    import concourse.bacc as bacc
    import concourse.mybir as mybir
    from concourse import tile

    f32 = mybir.dt.float32
    bf16 = mybir.dt.bfloat16
    Alu = mybir.AluOpType
    CCH = 64                      # channels per chunk
    CC = C // CCH                 # chunks per (b, hc)
    HCN = H // 128                # 4 h-chunks (2 bins each)

    nc = bacc.Bacc(None, target_bir_lowering=False)
    x_in = nc.declare_dram_parameter("x", [B_LOC, C, H, W], f32, isOutput=False)
    out = nc.declare_dram_parameter("out", [B_LOC, C, H_OUT, W_OUT], f32, isOutput=True)

    with tile.TileContext(nc) as tc:
        with (
            tc.tile_pool(name="pbp", bufs=3 if n_dve else 6) as pbp,
            tc.tile_pool(name="pqp", bufs=2 if n_dve else 4) as pqp,
            tc.tile_pool(name="dxp", bufs=2) as dxp,
            tc.tile_pool(name="dqp", bufs=1) as dqp,
            tc.tile_pool(name="ptp", bufs=3) as ptp,
            tc.tile_pool(name="psx", bufs=2, space="PSUM") as psx,
            tc.tile_pool(name="pso", bufs=2, space="PSUM") as pso,
            tc.tile_pool(name="op", bufs=1) as op,
        ):
            oacc = op.tile([C, B_LOC, H_OUT, W_OUT], f32, tag="oacc")
            eps_t = op.tile([C, 1], f32, tag="eps")
            nc.vector.memset(eps_t[:], float(EPS))
            # selector constants
            sel2f = op.tile([128, 2], f32, tag="sel2f")
            nc.vector.memset(sel2f[:], 1.0)
            # keep iff 0 <= p - 64*j <= 63  (i.e. j == p // 64)
            nc.gpsimd.affine_select(
                out=sel2f[:], in_=sel2f[:], pattern=[[-KH, 2]],
                compare_op=Alu.is_ge, fill=0.0, base=0, channel_multiplier=1,
            )
            nc.gpsimd.affine_select(
                out=sel2f[:], in_=sel2f[:], pattern=[[KH, 2]],
                compare_op=Alu.is_ge, fill=0.0, base=KH - 1, channel_multiplier=-1,
            )
            sel2b = op.tile([128, 2], bf16, tag="sel2b")
            nc.vector.tensor_copy(sel2b[:], sel2f[:])
            kwsel = op.tile([128, W_OUT], f32, tag="kwsel")
            nc.vector.memset(kwsel[:], 1.0)
            # keep iff 0 <= p - 8*j <= 7  (i.e. j == p // 8)
            nc.gpsimd.affine_select(
                out=kwsel[:], in_=kwsel[:], pattern=[[-KW, W_OUT]],
                compare_op=Alu.is_ge, fill=0.0, base=0, channel_multiplier=1,
            )
            nc.gpsimd.affine_select(
                out=kwsel[:], in_=kwsel[:], pattern=[[KW, W_OUT]],
                compare_op=Alu.is_ge, fill=0.0, base=KW - 1, channel_multiplier=-1,
            )

            dve_sel = {3: (1, 4, 6), 2: (1, 5), 1: (3,), 0: (),
                       4: (1, 3, 4, 6)}[n_dve]
            import contextlib
            loop_cm = (tc.For_i(0, loop_reps, 1) if loop_reps > 1
                       else contextlib.nullcontext())
            with loop_cm:
              for _ in range(reps):
                for b in range(B_LOC):
                    for hc in range(HCN):
                        if b * HCN + hc in dve_sel and variant == "full":
                            # fast p=c loads + DVE segmented reduces
                            for ih in (2 * hc, 2 * hc + 1):
                                xt = dxp.tile([C, KH, W], f32, tag="xtf")
                                nc.sync.dma_start(
                                    out=xt[:],
                                    in_=x_in[b, :, ih * KH:(ih + 1) * KH, :],
                                )
                                sqf = dqp.tile([C, KH, W], f32, tag="sqf")
                                nc.scalar.activation(
                                    sqf[:], xt[:],
                                    mybir.ActivationFunctionType.Square,
                                )
                                s1 = ptp.tile([C, W], f32, tag="s1")
                                s2 = ptp.tile([C, W], f32, tag="s2")
                                if gp_dve:
                                    # sum-x on GpSimd: in-place log-fold on xt
                                    # (safe: Square already consumed xt)
                                    hh = KH // 2
                                    while hh >= 1:
                                        dst = s1[:] if hh == 1 else xt[:, 0:hh, :]
                                        nc.gpsimd.tensor_add(
                                            dst, xt[:, 0:hh, :] if hh > 1 else xt[:, 0, :],
                                            xt[:, hh:2 * hh, :] if hh > 1 else xt[:, 1, :],
                                        )
                                        hh //= 2
                                else:
                                    nc.vector.reduce_sum(
                                        out=s1[:], in_=xt[:].transpose([0, 2, 1]),
                                        axis=mybir.AxisListType.X,
                                    )
                                nc.vector.reduce_sum(
                                    out=s2[:], in_=sqf[:].transpose([0, 2, 1]),
                                    axis=mybir.AxisListType.X,
                                )
                                meanv = ptp.tile([C, W], f32, tag="meanv")
                                m2v = ptp.tile([C, W], f32, tag="m2v")
                                varv = ptp.tile([C, W], f32, tag="varv")
                                stds = ptp.tile([C, W], f32, tag="stds")
                                nc.vector.tensor_scalar_mul(meanv[:], s1[:], 1.0 / KH)
                                nc.vector.tensor_mul(m2v[:], meanv[:], meanv[:])
                                nc.vector.scalar_tensor_tensor(
                                    out=varv[:], in0=s2[:], scalar=1.0 / KH,
                                    in1=m2v[:], op0=Alu.mult, op1=Alu.subtract,
                                )
                                nc.scalar.activation(
                                    stds[:], varv[:],
                                    mybir.ActivationFunctionType.Sqrt,
                                    bias=eps_t[:], scale=1.0,
                                )
                                nc.vector.reduce_sum(
                                    out=oacc[:, b, ih, :],
                                    in_=stds[:].rearrange("p (g k) -> p g k", k=KW),
                                    axis=mybir.AxisListType.X,
                                )
                            continue
                        ps_o = pso.tile([128, 2, W_OUT], f32, tag="ps_o")
                        # one cast-during-DMA load of the whole [128h, C, W]
                        # unit (8 MiB HBM-side, 4 MiB bf16 SBUF-side, SWDGE)
                        if variant == "dmah":
                            # HWDGE fp32 loads, same transposed 512B-run pattern
                            pxf = pbp.tile([128, C // 2, W], f32, tag="pxf")
                            for dh in range(2):
                                nc.sync.dma_start(
                                    out=pxf[:],
                                    in_=x_in[
                                        b, dh * (C // 2):(dh + 1) * (C // 2),
                                        hc * 128:(hc + 1) * 128, :,
                                    ].transpose([1, 0, 2]),
                                )
                            continue
                        for cc in range(CC):
                            pxb = pbp.tile([128, CCH, W], bf16, tag="pxb")
                            nc.gpsimd.dma_start(
                                out=pxb[:],
                                in_=x_in[
                                    b, cc * CCH:(cc + 1) * CCH,
                                    hc * 128:(hc + 1) * 128, :,
                                ].transpose([1, 0, 2]),
                                single_packet=single_packet,
                            )
                            if variant == "dma":
                                continue
                            psqb = pqp.tile([128, CCH, W], bf16, tag="psqb")
                            if sq_alt and cc % 2 == 0:
                                nc.vector.tensor_mul(psqb[:], pxb[:], pxb[:])
                            else:
                                nc.scalar.activation(
                                    psqb[:], pxb[:],
                                    mybir.ActivationFunctionType.Square,
                                )
                            ps_x = psx.tile([128, 2 * CCH], f32, tag="ps_x")
                            ps_q = psx.tile([128, 2 * CCH], f32, tag="ps_q")
                            for c0 in range(CCH):
                                nc.tensor.matmul(
                                    ps_x[:, 2 * c0:2 * c0 + 2],
                                    pxb[:, c0, :], sel2b[:],
                                    start=True, stop=True,
                                )
                                nc.tensor.matmul(
                                    ps_q[:, 2 * c0:2 * c0 + 2],
                                    psqb[:, c0, :], sel2b[:],
                                    start=True, stop=True,
                                )
                            mean = ptp.tile([128, 2 * CCH], f32, tag="mean")
                            m2 = ptp.tile([128, 2 * CCH], f32, tag="m2")
                            var = ptp.tile([128, 2 * CCH], f32, tag="var")
                            pstd = ptp.tile([128, 2, CCH], f32, tag="pstd")
                            nc.vector.tensor_scalar_mul(mean[:], ps_x[:], 1.0 / KH)
                            nc.vector.tensor_mul(m2[:], mean[:], mean[:])
                            nc.vector.scalar_tensor_tensor(
                                out=var[:], in0=ps_q[:], scalar=1.0 / KH,
                                in1=m2[:], op0=Alu.mult, op1=Alu.subtract,
                            )
                            nc.scalar.activation(
                                pstd[:].transpose([0, 2, 1]),
                                var[:].rearrange("p (c t) -> p c t", t=2),
                                mybir.ActivationFunctionType.Sqrt,
                                bias=eps_t[:], scale=1.0,
                            )
                            for bin_ in range(2):
                                nc.tensor.matmul(
                                    ps_o[cc * CCH:(cc + 1) * CCH, bin_, :],
                                    pstd[:, bin_, :], kwsel[:],
                                    start=True, stop=True,
                                )
                        if variant == "dma":
                            continue
                        nc.vector.tensor_copy(
                            oacc[:, b, 2 * hc:2 * hc + 2, :], ps_o[:],
                        )
            if variant in ("dma", "dmah"):
                nc.vector.memset(oacc[:], 0.0)
            nc.sync.dma_start(out=out.transpose([1, 0, 2, 3]), in_=oacc[:])
    nc.finalize()
    return nc


def _build_pec(variant="full", n_dve=3, single_packet=False, loop_reps=1,
               hw_dve_loads=True, fold_dve=False):
    """PE reduction path with host-pre-transposed input for contiguous loads.

    Second dram input xt[b, hc, h, c, w] = x[b, c, 128*hc + h, w] lets the
    PE-path units load [128h, 64c, 128w] tiles with 32 KiB-contiguous
    per-partition runs (SWDGE fp32->bf16 cast during DMA) instead of the
    512 B-run transposed pattern.  dve_sel units read the natural-layout x
    with contiguous HWDGE loads as before.  Each element of the input is
    read exactly once per rep (from one of the two copies).
    """
    import contextlib

    import concourse.bacc as bacc
    import concourse.mybir as mybir
    from concourse import tile

    f32 = mybir.dt.float32
    bf16 = mybir.dt.bfloat16
    Alu = mybir.AluOpType
    CCH = 64
    CC = C // CCH
    HCN = H // 128

    nc = bacc.Bacc(None, target_bir_lowering=False)
    x_in = nc.declare_dram_parameter("x", [B_LOC, C, H, W], f32, isOutput=False)
    xt_in = nc.declare_dram_parameter("xt", [B_LOC, HCN, 128, C, W], f32,
                                      isOutput=False)
    out = nc.declare_dram_parameter("out", [B_LOC, C, H_OUT, W_OUT], f32, isOutput=True)

    with tile.TileContext(nc) as tc:
        with (
            tc.tile_pool(name="pbp", bufs=3 if n_dve else 6) as pbp,
            tc.tile_pool(name="pqp", bufs=2 if n_dve else 4) as pqp,
            tc.tile_pool(name="dxp", bufs=2) as dxp,
            tc.tile_pool(name="dqp", bufs=1) as dqp,
            tc.tile_pool(name="ptp", bufs=3) as ptp,
            tc.tile_pool(name="psx", bufs=2, space="PSUM") as psx,
            tc.tile_pool(name="pso", bufs=2, space="PSUM") as pso,
            tc.tile_pool(name="op", bufs=1) as op,
        ):
            oacc = op.tile([C, B_LOC, H_OUT, W_OUT], f32, tag="oacc")
            eps_t = op.tile([C, 1], f32, tag="eps")
            nc.vector.memset(eps_t[:], float(EPS))
            sel2f = op.tile([128, 2], f32, tag="sel2f")
            nc.vector.memset(sel2f[:], 1.0)
            nc.gpsimd.affine_select(
                out=sel2f[:], in_=sel2f[:], pattern=[[-KH, 2]],
                compare_op=Alu.is_ge, fill=0.0, base=0, channel_multiplier=1,
            )
            nc.gpsimd.affine_select(
                out=sel2f[:], in_=sel2f[:], pattern=[[KH, 2]],
                compare_op=Alu.is_ge, fill=0.0, base=KH - 1, channel_multiplier=-1,
            )
            sel2b = op.tile([128, 2], bf16, tag="sel2b")
            nc.vector.tensor_copy(sel2b[:], sel2f[:])
            kwsel = op.tile([128, W_OUT], f32, tag="kwsel")
            nc.vector.memset(kwsel[:], 1.0)
            nc.gpsimd.affine_select(
                out=kwsel[:], in_=kwsel[:], pattern=[[-KW, W_OUT]],
                compare_op=Alu.is_ge, fill=0.0, base=0, channel_multiplier=1,
            )
            nc.gpsimd.affine_select(
                out=kwsel[:], in_=kwsel[:], pattern=[[KW, W_OUT]],
                compare_op=Alu.is_ge, fill=0.0, base=KW - 1, channel_multiplier=-1,
            )

            dve_sel = {3: (1, 4, 6), 2: (1, 5), 1: (3,), 0: (),
                       4: (1, 3, 4, 6)}[n_dve]
            loop_cm = (tc.For_i(0, loop_reps, 1) if loop_reps > 1
                       else contextlib.nullcontext())
            with loop_cm:
                for b in range(B_LOC):
                    for hc in range(HCN):
                        if (b * HCN + hc in dve_sel and variant == "full"
                                and fold_dve):
                            # unit-stride bf16 log-fold reduction on DVE
                            for ih in (2 * hc, 2 * hc + 1):
                                xb = pbp.tile([128, KH, W], bf16, tag="pxb")
                                nc.gpsimd.dma_start(
                                    out=xb[:],
                                    in_=x_in[b, :, ih * KH:(ih + 1) * KH, :],
                                    single_packet=single_packet,
                                )
                                sqb = pqp.tile([128, KH, W], bf16, tag="psqb")
                                nc.scalar.activation(
                                    sqb[:], xb[:],
                                    mybir.ActivationFunctionType.Square,
                                )
                                s1 = ptp.tile([C, W], f32, tag="s1")
                                s2 = ptp.tile([C, W], f32, tag="s2")
                                for src, dst, ftag in ((xb, s1, "fx"),
                                                       (sqb, s2, "fq")):
                                    f1 = dxp.tile([C, KH // 2, W], bf16,
                                                  tag=ftag)
                                    nc.vector.tensor_add(
                                        f1[:], src[:, 0:32, :], src[:, 32:64, :])
                                    nc.vector.tensor_add(
                                        f1[:, 0:16, :], f1[:, 0:16, :],
                                        f1[:, 16:32, :])
                                    nc.vector.tensor_add(
                                        f1[:, 0:8, :], f1[:, 0:8, :],
                                        f1[:, 8:16, :])
                                    f2 = dqp.tile([C, 4, W], f32,
                                                  tag=ftag + "f")
                                    nc.vector.tensor_add(
                                        f2[:], f1[:, 0:4, :], f1[:, 4:8, :])
                                    nc.vector.tensor_add(
                                        f2[:, 0:2, :], f2[:, 0:2, :],
                                        f2[:, 2:4, :])
                                    nc.vector.tensor_add(
                                        dst[:], f2[:, 0, :], f2[:, 1, :])
                                meanv = ptp.tile([C, W], f32, tag="meanv")
                                m2v = ptp.tile([C, W], f32, tag="m2v")
                                varv = ptp.tile([C, W], f32, tag="varv")
                                stds = ptp.tile([C, W], f32, tag="stds")
                                nc.vector.tensor_scalar_mul(
                                    meanv[:], s1[:], 1.0 / KH)
                                nc.vector.tensor_mul(m2v[:], meanv[:], meanv[:])
                                nc.vector.scalar_tensor_tensor(
                                    out=varv[:], in0=s2[:], scalar=1.0 / KH,
                                    in1=m2v[:], op0=Alu.mult, op1=Alu.subtract,
                                )
                                nc.scalar.activation(
                                    stds[:], varv[:],
                                    mybir.ActivationFunctionType.Sqrt,
                                    bias=eps_t[:], scale=1.0,
                                )
                                nc.vector.reduce_sum(
                                    out=oacc[:, b, ih, :],
                                    in_=stds[:].rearrange(
                                        "p (g k) -> p g k", k=KW),
                                    axis=mybir.AxisListType.X,
                                )
                            continue
                        if b * HCN + hc in dve_sel and variant == "full":
                            for ih in (2 * hc, 2 * hc + 1):
                                xt = dxp.tile([C, KH, W], f32, tag="xtf")
                                if hw_dve_loads:
                                    nc.sync.dma_start(
                                        out=xt[:],
                                        in_=x_in[b, :, ih * KH:(ih + 1) * KH, :],
                                    )
                                else:
                                    nc.gpsimd.dma_start(
                                        out=xt[:],
                                        in_=x_in[b, :, ih * KH:(ih + 1) * KH, :],
                                        single_packet=single_packet,
                                    )
                                sqf = dqp.tile([C, KH, W], f32, tag="sqf")
                                nc.scalar.activation(
                                    sqf[:], xt[:],
                                    mybir.ActivationFunctionType.Square,
                                )
                                s1 = ptp.tile([C, W], f32, tag="s1")
                                s2 = ptp.tile([C, W], f32, tag="s2")
                                nc.vector.reduce_sum(
                                    out=s1[:], in_=xt[:].transpose([0, 2, 1]),
                                    axis=mybir.AxisListType.X,
                                )
                                nc.vector.reduce_sum(
                                    out=s2[:], in_=sqf[:].transpose([0, 2, 1]),
                                    axis=mybir.AxisListType.X,
                                )
                                meanv = ptp.tile([C, W], f32, tag="meanv")
                                m2v = ptp.tile([C, W], f32, tag="m2v")
                                varv = ptp.tile([C, W], f32, tag="varv")
                                stds = ptp.tile([C, W], f32, tag="stds")
                                nc.vector.tensor_scalar_mul(meanv[:], s1[:], 1.0 / KH)
                                nc.vector.tensor_mul(m2v[:], meanv[:], meanv[:])
                                nc.vector.scalar_tensor_tensor(
                                    out=varv[:], in0=s2[:], scalar=1.0 / KH,
                                    in1=m2v[:], op0=Alu.mult, op1=Alu.subtract,
                                )
                                nc.scalar.activation(
                                    stds[:], varv[:],
                                    mybir.ActivationFunctionType.Sqrt,
                                    bias=eps_t[:], scale=1.0,
                                )
                                nc.vector.reduce_sum(
                                    out=oacc[:, b, ih, :],
                                    in_=stds[:].rearrange("p (g k) -> p g k", k=KW),
                                    axis=mybir.AxisListType.X,
                                )
                            continue
                        ps_o = pso.tile([128, 2, W_OUT], f32, tag="ps_o")
                        for cc in range(CC):
                            pxb = pbp.tile([128, CCH, W], bf16, tag="pxb")
                            nc.gpsimd.dma_start(
                                out=pxb[:],
                                in_=xt_in[b, hc, :, cc * CCH:(cc + 1) * CCH, :],
                                single_packet=single_packet,
                            )
                            if variant == "dma":
                                continue
                            psqb = pqp.tile([128, CCH, W], bf16, tag="psqb")
                            nc.scalar.activation(
                                psqb[:], pxb[:],
                                mybir.ActivationFunctionType.Square,
                            )
                            ps_x = psx.tile([128, 2 * CCH], f32, tag="ps_x")
                            ps_q = psx.tile([128, 2 * CCH], f32, tag="ps_q")
                            for c0 in range(CCH):
                                nc.tensor.matmul(
                                    ps_x[:, 2 * c0:2 * c0 + 2],
                                    pxb[:, c0, :], sel2b[:],
                                    start=True, stop=True,
                                )
                                nc.tensor.matmul(
                                    ps_q[:, 2 * c0:2 * c0 + 2],
                                    psqb[:, c0, :], sel2b[:],
                                    start=True, stop=True,
                                )
                            mean = ptp.tile([128, 2 * CCH], f32, tag="mean")
                            m2 = ptp.tile([128, 2 * CCH], f32, tag="m2")
                            var = ptp.tile([128, 2 * CCH], f32, tag="var")
                            pstd = ptp.tile([128, 2, CCH], f32, tag="pstd")
                            nc.vector.tensor_scalar_mul(mean[:], ps_x[:], 1.0 / KH)
                            nc.vector.tensor_mul(m2[:], mean[:], mean[:])
                            nc.vector.scalar_tensor_tensor(
                                out=var[:], in0=ps_q[:], scalar=1.0 / KH,
                                in1=m2[:], op0=Alu.mult, op1=Alu.subtract,
                            )
                            nc.scalar.activation(
                                pstd[:].transpose([0, 2, 1]),
                                var[:].rearrange("p (c t) -> p c t", t=2),
                                mybir.ActivationFunctionType.Sqrt,
                                bias=eps_t[:], scale=1.0,
                            )
                            for bin_ in range(2):
                                nc.tensor.matmul(
                                    ps_o[cc * CCH:(cc + 1) * CCH, bin_, :],
                                    pstd[:, bin_, :], kwsel[:],
                                    start=True, stop=True,
                                )
                        if variant == "dma":
                            continue
                        nc.vector.tensor_copy(
                            oacc[:, b, 2 * hc:2 * hc + 2, :], ps_o[:],
                        )
            if variant == "dma":
                nc.vector.memset(oacc[:], 0.0)
            nc.sync.dma_start(out=out.transpose([1, 0, 2, 3]), in_=oacc[:])
    nc.finalize()
    return nc


def make_xt(x_loc):
    """Host-side transpose for _build_pec: [C,H,W] -> [HCN,128,C,W] per b."""
    B_l = x_loc.shape[0]
    return np.ascontiguousarray(
        x_loc.reshape(B_l, C, H // 128, 128, W).transpose(0, 2, 3, 1, 4)
    )


# unit u = (b, hc) with b = u // 4, hc = u % 4; 8 units of [*, 128, *] rows.
# FOLD_UNITS get p=c layout (DVE fold reduction), the rest p=h (PE reduce).
# Last unit (7) is a PE unit with quarter-granular DMAs (short tail).
FOLD_UNITS = {3: (1, 4, 6), 2: (3, 6), 1: (6,), 0: (), 4: (1, 3, 5, 6),
              5: (0, 2, 4, 5, 6)}


def make_v2_inputs(x_loc, n_dve):
    """Pack per-unit slabs: xf [Uf, C, 128, W] (fold), xp [Up, 128, C, W]."""
    fold = FOLD_UNITS[n_dve]
    xf_l, xp_l = [], []
    for u in range(8):
        b, hc = u // 4, u % 4
        slab = x_loc[b, :, hc * 128:(hc + 1) * 128, :]
        if u in fold:
            xf_l.append(slab)
        else:
            xp_l.append(slab.transpose(1, 0, 2))
    xf = (np.ascontiguousarray(np.stack(xf_l)) if xf_l
          else np.zeros((1, C, 128, W), np.float32))
    xp = (np.ascontiguousarray(np.stack(xp_l)) if xp_l
          else np.zeros((1, 128, C, W), np.float32))
    return {"xf": xf, "xp": xp}


def _build_v2(variant="full", n_dve=3, single_packet=False, loop_reps=1,
              xb_bufs=6, sq_bufs=3):
    """Final layout: one contiguous 8 MiB SWDGE bf16-cast DMA per unit.

    PE units ([128h, C, W] slabs): ACT square -> per-channel ldweights/matmul
    pairs against a 2-column bin selector -> var/std on DVE/ACT -> kw-sum via
    a second tiny matmul.  Fold units ([C, 128h, W] slabs): ACT square ->
    unit-stride bf16 in-place log-fold on DVE for both chains -> var/std ->
    kw reduce_sum.  Engines: DMA ~194us, PE ~27us/unit, DVE ~21us/unit,
    ACT ~13.6us/unit."""
    import contextlib

    import concourse.bacc as bacc
    import concourse.mybir as mybir
    from concourse import tile

    f32 = mybir.dt.float32
    bf16 = mybir.dt.bfloat16
    Alu = mybir.AluOpType

    fold_units = FOLD_UNITS[n_dve]
    n_pe = 8 - len(fold_units)
    nc = bacc.Bacc(None, target_bir_lowering=False)
    xf_in = nc.declare_dram_parameter(
        "xf", [max(len(fold_units), 1), C, 128, W], f32, isOutput=False)
    xp_in = nc.declare_dram_parameter(
        "xp", [max(n_pe, 1), 128, C, W], f32, isOutput=False)
    out = nc.declare_dram_parameter("out", [B_LOC, C, H_OUT, W_OUT], f32,
                                    isOutput=True)

    with tile.TileContext(nc) as tc:
        with (
            tc.tile_pool(name="xbp", bufs=xb_bufs) as xbp,
            tc.tile_pool(name="sqp", bufs=sq_bufs) as sqp,
            tc.tile_pool(name="ffp", bufs=2) as ffp,
            tc.tile_pool(name="ftp", bufs=1) as ftp,
            tc.tile_pool(name="ptp", bufs=3) as ptp,
            tc.tile_pool(name="psx", bufs=2, space="PSUM") as psx,
            tc.tile_pool(name="pso", bufs=2, space="PSUM") as pso,
            tc.tile_pool(name="op", bufs=1) as op,
        ):
            oacc = op.tile([C, B_LOC, H_OUT, W_OUT], f32, tag="oacc")
            eps_t = op.tile([C, 1], f32, tag="eps")
            nc.vector.memset(eps_t[:], float(EPS))
            sel2f = op.tile([128, 2], f32, tag="sel2f")
            nc.vector.memset(sel2f[:], 1.0)
            nc.gpsimd.affine_select(
                out=sel2f[:], in_=sel2f[:], pattern=[[-KH, 2]],
                compare_op=Alu.is_ge, fill=0.0, base=0, channel_multiplier=1,
            )
            nc.gpsimd.affine_select(
                out=sel2f[:], in_=sel2f[:], pattern=[[KH, 2]],
                compare_op=Alu.is_ge, fill=0.0, base=KH - 1,
                channel_multiplier=-1,
            )
            sel2b = op.tile([128, 2], bf16, tag="sel2b")
            nc.vector.tensor_copy(sel2b[:], sel2f[:])
            kwsel = op.tile([128, W_OUT], f32, tag="kwsel")
            nc.vector.memset(kwsel[:], 1.0)
            nc.gpsimd.affine_select(
                out=kwsel[:], in_=kwsel[:], pattern=[[-KW, W_OUT]],
                compare_op=Alu.is_ge, fill=0.0, base=0, channel_multiplier=1,
            )
            nc.gpsimd.affine_select(
                out=kwsel[:], in_=kwsel[:], pattern=[[KW, W_OUT]],
                compare_op=Alu.is_ge, fill=0.0, base=KW - 1,
                channel_multiplier=-1,
            )

            loop_cm = (tc.For_i(0, loop_reps, 1) if loop_reps > 1
                       else contextlib.nullcontext())
            with loop_cm:
                fi = pi = 0
                for u in range(8):
                    b, hc = u // 4, u % 4
                    if u in fold_units:
                        src, idx = xf_in, fi
                        fi += 1
                    else:
                        src, idx = xp_in, pi
                        pi += 1
                    tail = u == 7 and u not in fold_units and variant == "full"
                    halves = []
                    for hv in range(2):
                        xb = xbp.tile([128, 64, W], bf16, tag="xb")
                        nsub = 2 if tail else 1
                        for q in range(nsub):
                            w0 = 64 // nsub
                            nc.gpsimd.dma_start(
                                out=xb[:, w0 * q:w0 * (q + 1), :],
                                in_=src[idx, :,
                                        64 * hv + w0 * q:64 * hv + w0 * (q + 1),
                                        :],
                                single_packet=single_packet,
                            )
                        if variant == "dma":
                            continue
                        sqb = sqp.tile([128, 64, W], bf16, tag="sqb")
                        for q in range(nsub):
                            w0 = 64 // nsub
                            nc.scalar.activation(
                                sqb[:, w0 * q:w0 * (q + 1), :],
                                xb[:, w0 * q:w0 * (q + 1), :],
                                mybir.ActivationFunctionType.Square,
                            )
                        halves.append((xb, sqb))
                    if variant == "dma":
                        continue
                    if u in fold_units:
                        # halves are [C, 64h, W] = rows ih=2hc, 2hc+1
                        for half, ih in ((0, 2 * hc), (1, 2 * hc + 1)):
                            s1 = ptp.tile([C, W], f32, tag="s1")
                            s2 = ptp.tile([C, W], f32, tag="s2")
                            for t, dst, ftag in ((halves[half][0], s1, "fx"),
                                                 (halves[half][1], s2, "fq")):
                                v = t[:]
                                f1 = ffp.tile([C, 32, W], bf16, tag=ftag)
                                nc.vector.tensor_add(
                                    f1[:], v[:, 0:32, :], v[:, 32:64, :])
                                nc.vector.tensor_add(
                                    f1[:, 0:16, :], f1[:, 0:16, :],
                                    f1[:, 16:32, :])
                                nc.vector.tensor_add(
                                    f1[:, 0:8, :], f1[:, 0:8, :],
                                    f1[:, 8:16, :])
                                f2 = ftp.tile([C, 4, W], f32, tag=ftag + "f")
                                nc.vector.tensor_add(
                                    f2[:], f1[:, 0:4, :], f1[:, 4:8, :])
                                nc.vector.tensor_add(
                                    f2[:, 0:2, :], f2[:, 0:2, :],
                                    f2[:, 2:4, :])
                                nc.vector.tensor_add(
                                    dst[:], f2[:, 0, :], f2[:, 1, :])
                            meanv = ptp.tile([C, W], f32, tag="meanv")
                            m2v = ptp.tile([C, W], f32, tag="m2v")
                            varv = ptp.tile([C, W], f32, tag="varv")
                            stds = ptp.tile([C, W], f32, tag="stds")
                            nc.vector.tensor_scalar_mul(
                                meanv[:], s1[:], 1.0 / KH)
                            nc.vector.tensor_mul(m2v[:], meanv[:], meanv[:])
                            nc.vector.scalar_tensor_tensor(
                                out=varv[:], in0=s2[:], scalar=1.0 / KH,
                                in1=m2v[:], op0=Alu.mult, op1=Alu.subtract,
                            )
                            nc.scalar.activation(
                                stds[:], varv[:],
                                mybir.ActivationFunctionType.Sqrt,
                                bias=eps_t[:], scale=1.0,
                            )
                            nc.vector.reduce_sum(
                                out=oacc[:, b, ih, :],
                                in_=stds[:].rearrange("p (g k) -> p g k",
                                                      k=KW),
                                axis=mybir.AxisListType.X,
                            )
                    else:
                        # xb/sqb are [128h, C, W]; PE selector reduction
                        ps_o = pso.tile([128, 2, W_OUT], f32, tag="ps_o")
                        ps_x = psx.tile([128, 2 * C], f32, tag="ps_x")
                        ps_q = psx.tile([128, 2 * C], f32, tag="ps_q")
                        for c0 in range(C):
                            xh, sh = halves[c0 // 64]
                            nc.tensor.matmul(
                                ps_x[:, 2 * c0:2 * c0 + 2],
                                xh[:, c0 % 64, :], sel2b[:],
                                start=True, stop=True,
                            )
                            nc.tensor.matmul(
                                ps_q[:, 2 * c0:2 * c0 + 2],
                                sh[:, c0 % 64, :], sel2b[:],
                                start=True, stop=True,
                            )
                        mean = ptp.tile([128, 2 * C], f32, tag="mean")
                        m2 = ptp.tile([128, 2 * C], f32, tag="m2")
                        var = ptp.tile([128, 2 * C], f32, tag="var")
                        pstd = ptp.tile([128, 2, C], f32, tag="pstd")
                        nc.vector.tensor_scalar_mul(mean[:], ps_x[:], 1.0 / KH)
                        nc.vector.tensor_mul(m2[:], mean[:], mean[:])
                        nc.vector.scalar_tensor_tensor(
                            out=var[:], in0=ps_q[:], scalar=1.0 / KH,
                            in1=m2[:], op0=Alu.mult, op1=Alu.subtract,
                        )
                        nc.scalar.activation(
                            pstd[:].transpose([0, 2, 1]),
                            var[:].rearrange("p (c t) -> p c t", t=2),
                            mybir.ActivationFunctionType.Sqrt,
                            bias=eps_t[:], scale=1.0,
                        )
                        for bin_ in range(2):
                            nc.tensor.matmul(
                                ps_o[:, bin_, :],
                                pstd[:, bin_, :], kwsel[:],
                                start=True, stop=True,
                            )
                        nc.vector.tensor_copy(
                            oacc[:, b, 2 * hc:2 * hc + 2, :], ps_o[:],
                        )
            if variant == "dma":
                nc.vector.memset(oacc[:], 0.0)
            nc.sync.dma_start(out=out.transpose([1, 0, 2, 3]), in_=oacc[:])
    nc.finalize()
    return nc


def _build_fold(reps=1, variant="full", n_dve_sq=1, gp_fold=True, loop_reps=1):
    """p=c layout everywhere (fast 32KiB-contiguous loads, bf16 cast in DMA).

    Per (b, ih) tile [C, KH=64, W] bf16: square on ACT (a few tiles' squares
    go to DVE), then segmented sum over the 64-row bin via log2 folding
    (tensor_add): the first fold level (half the work) runs on GpSimd, the
    rest on DVE at bf16 2x.  Final level accumulates to fp32.  var/std/kw-sum
    as usual.
    """
    import concourse.bacc as bacc
    import concourse.mybir as mybir
    from concourse import tile

    f32 = mybir.dt.float32
    bf16 = mybir.dt.bfloat16
    Alu = mybir.AluOpType

    nc = bacc.Bacc(None, target_bir_lowering=False)
    x_in = nc.declare_dram_parameter("x", [B_LOC, C, H, W], f32, isOutput=False)
    out = nc.declare_dram_parameter("out", [B_LOC, C, H_OUT, W_OUT], f32, isOutput=True)

    with tile.TileContext(nc) as tc:
        with (
            tc.tile_pool(name="xbp", bufs=3) as xbp,
            tc.tile_pool(name="sqp", bufs=2) as sqp,
            tc.tile_pool(name="fp", bufs=2) as fp,
            tc.tile_pool(name="tp", bufs=3) as tp,
            tc.tile_pool(name="op", bufs=1) as op,
        ):
            oacc = op.tile([C, B_LOC, H_OUT, W_OUT], f32, tag="oacc")
            eps_t = op.tile([C, 1], f32, tag="eps")
            nc.vector.memset(eps_t[:], float(EPS))

            def fold_sum(src, out_s1, tag):
                # src [C, 64, W] bf16 -> out_s1 [C, W] f32 (sum over axis 1).
                # First level on GpSimd, mid levels bf16@2x on DVE, tail fp32.
                h = KH // 2
                eng = nc.gpsimd if gp_fold else nc.vector
                cur = fp.tile([C, h, W], bf16, tag=f"{tag}{h}")
                eng.tensor_add(cur[:], src[:, 0:h, :], src[:, h:2 * h, :])
                while h > 8:
                    h //= 2
                    nxt = fp.tile([C, h, W], bf16, tag=f"{tag}{h}")
                    nc.vector.tensor_add(nxt[:], cur[:, 0:h, :], cur[:, h:2 * h, :])
                    cur = nxt
                while h > 2:
                    h //= 2
                    nxt = fp.tile([C, h, W], f32, tag=f"{tag}f{h}")
                    nc.vector.tensor_add(nxt[:], cur[:, 0:h, :], cur[:, h:2 * h, :])
                    cur = nxt
                nc.vector.tensor_add(out_s1[:], cur[:, 0, :], cur[:, 1, :])

            ti = 0
            import contextlib
            loop_cm = (tc.For_i(0, loop_reps, 1) if loop_reps > 1
                       else contextlib.nullcontext())
            with loop_cm:
             for _ in range(reps):
                for b in range(B_LOC):
                    for ih in range(H_OUT):
                        xb = xbp.tile([C, KH, W], bf16, tag="xb")
                        nc.gpsimd.dma_start(
                            out=xb[:], in_=x_in[b, :, ih * KH:(ih + 1) * KH, :],
                        )
                        if variant == "dma":
                            continue
                        sqb = sqp.tile([C, KH, W], bf16, tag="sqb")
                        if ti % H_OUT < n_dve_sq:
                            nc.vector.tensor_mul(sqb[:], xb[:], xb[:])
                        else:
                            nc.scalar.activation(
                                sqb[:], xb[:], mybir.ActivationFunctionType.Square,
                            )
                        ti += 1
                        s1 = tp.tile([C, W], f32, tag="s1")
                        s2 = tp.tile([C, W], f32, tag="s2")
                        fold_sum(xb, s1, "fx")
                        fold_sum(sqb, s2, "fq")
                        meanv = tp.tile([C, W], f32, tag="meanv")
                        m2v = tp.tile([C, W], f32, tag="m2v")
                        varv = tp.tile([C, W], f32, tag="varv")
                        stds = tp.tile([C, W], f32, tag="stds")
                        nc.vector.tensor_scalar_mul(meanv[:], s1[:], 1.0 / KH)
                        nc.vector.tensor_mul(m2v[:], meanv[:], meanv[:])
                        nc.vector.scalar_tensor_tensor(
                            out=varv[:], in0=s2[:], scalar=1.0 / KH,
                            in1=m2v[:], op0=Alu.mult, op1=Alu.subtract,
                        )
                        nc.scalar.activation(
                            stds[:], varv[:], mybir.ActivationFunctionType.Sqrt,
                            bias=eps_t[:], scale=1.0,
                        )
                        nc.vector.reduce_sum(
                            out=oacc[:, b, ih, :],
                            in_=stds[:].rearrange("p (g k) -> p g k", k=KW),
                            axis=mybir.AxisListType.X,
                        )
            if variant == "dma":
                nc.vector.memset(oacc[:], 0.0)
            nc.sync.dma_start(out=out.transpose([1, 0, 2, 3]), in_=oacc[:])
    nc.finalize()
    return nc


def kernel(x):
    import os
    from concourse.bass_utils import run_bass_kernel_spmd

    x = np.ascontiguousarray(np.asarray(x, dtype=np.float32))
    assert x.shape == (B, C, H, W), x.shape

    impl = os.environ.get("KERNEL_IMPL", "v2_3")
    if _CACHE.get("impl") != impl:
        if impl.startswith("v2"):
            nd = int(impl.split("_")[1]) if "_" in impl else 3
            _CACHE["nc"] = _build_v2(n_dve=nd)
            _CACHE["n_dve"] = nd
        elif impl.startswith("pe"):
            rest = impl[2:]
            sq_alt = rest.endswith("s")
            if sq_alt:
                rest = rest[:-1]
            _CACHE["nc"] = _build_pe(n_dve=int(rest) if rest else 0, sq_alt=sq_alt,
                                     single_packet=True)
        elif impl.startswith("fold"):
            _CACHE["nc"] = _build_fold()
        else:
            _CACHE["nc"] = _build_bass()
        _CACHE["impl"] = impl
    nc = _CACHE["nc"]

    if impl.startswith("v2"):
        in_maps = [
            make_v2_inputs(x[i * B_LOC:(i + 1) * B_LOC], _CACHE["n_dve"])
            for i in range(N_CORES)
        ]
    else:
        in_maps = [{"x": x[i * B_LOC:(i + 1) * B_LOC]} for i in range(N_CORES)]
    last_err = None
    for _ in range(3):
        try:
            res = run_bass_kernel_spmd(nc, in_maps, core_ids=list(range(N_CORES)))
            break
        except Exception as e:  # transient axon/device hiccups
            last_err = e
    else:
        raise last_err
    return np.concatenate([np.asarray(r["out"]) for r in res.results], axis=0)



# revision 2
# speedup vs baseline: 1.9737x; 1.9737x over previous
"""AdaptiveStdPooling2d on 8 TRN2 NeuronCores.

Input  x: [16, 128, 512, 128] f32.  Output: [16, 128, 8, 16] f32.
out[b,c,i,j] = sum_{kw=0..7} std_h(x[b, c, 64*i:64*i+64, 8*j+kw])
with biased variance over the 64-row bin plus EPS=1e-14 inside sqrt.

Sharding: pure data parallel over batch B=16 -> 2 per core, no collectives.

The kernel computes in bf16 (as the previous fp32-HBM version already did
via cast-during-DMA), so the host pre-casts x to bf16 and the device reads
16 of the 32 MiB/core it used to — DMA floor ~94 us at ~358 GB/s/core.

Per core, the 8 slabs [128c, 128h, 128w] ("units", u = b*4 + hc) split:
  - 6 "PE units" (host-transposed to [128h, C, W]): square on ACT or DVE,
    then per-channel ldweights/matmul pairs against a [128,2] 0/1 bin
    selector — FWL weight loads make each pair ~27 ns, so the tensor
    engine does both segmented reductions at ~14 us/unit.  var/std from
    PSUM on DVE/ACT, kw-sum via a second tiny matmul against a [128,16]
    selector (which also lands the result back c-on-partitions).
  - 2 "fold units" (natural layout, read directly from the cast x):
    ACT square, then DVE log-fold (fresh-tile adds, bf16 deep, fp32 tail)
    into per-bin sums; a single batched var/sqrt/kw pass at the end.
Fold units sit early in program order so their batched tail overlaps the
PE units' stream; the last unit is a PE unit with a DVE square (fast tail).
"""

import contextlib

import numpy as np

B, C, H, W = 16, 128, 512, 128
N_CORES = 8
B_LOC = B // N_CORES          # 2 batches per core
H_OUT, W_OUT = 8, 16
KH, KW = H // H_OUT, W // W_OUT   # 64, 8
EPS = 1e-14

PE_UNITS = (1, 2, 3, 5, 6, 7)
SQ_DVE_UNITS = (5, 6, 7)

_CACHE = {}


def make_inputs(x16_loc, pe_units=PE_UNITS):
    """x16_loc [B_LOC, C, H, W] bf16 -> {"xn": ..., "xp": [NP,128,C,W]}."""
    xp_l = []
    for u in range(8):
        b, hc = u // 4, u % 4
        if u in pe_units:
            slab = x16_loc[b, :, hc * 128:(hc + 1) * 128, :]
            xp_l.append(slab.transpose(1, 0, 2))
    xp = (np.ascontiguousarray(np.stack(xp_l)) if xp_l
          else np.zeros((1, 128, C, W), x16_loc.dtype))
    xn = np.ascontiguousarray(x16_loc).reshape(B_LOC, C, 4, 2, KH, W)
    return {"xn": xn, "xp": xp}


def build(loop_reps=1, pe_units=PE_UNITS, sq_dve_units=SQ_DVE_UNITS):
    import concourse.bacc as bacc
    import concourse.mybir as mybir
    from concourse import tile

    f32 = mybir.dt.float32
    bf16 = mybir.dt.bfloat16
    Alu = mybir.AluOpType
    Act = mybir.ActivationFunctionType

    np_ = max(len(pe_units), 1)

    nc = bacc.Bacc(None, target_bir_lowering=False)
    xn_in = nc.declare_dram_parameter("xn", [B_LOC, C, 4, 2, KH, W], bf16,
                                      isOutput=False)
    xp_in = nc.declare_dram_parameter("xp", [np_, 128, C, W], bf16,
                                      isOutput=False)
    out = nc.declare_dram_parameter("out", [B_LOC, C, H_OUT, W_OUT], f32,
                                    isOutput=True)

    with tile.TileContext(nc) as tc:
        with (
            tc.tile_pool(name="xbp", bufs=4) as xbp,
            tc.tile_pool(name="sqp", bufs=3) as sqp,
            tc.tile_pool(name="glp", bufs=2) as glp,
            tc.tile_pool(name="ftp", bufs=2) as ftp,
            tc.tile_pool(name="vbp", bufs=1) as vbp,
            tc.tile_pool(name="ptp", bufs=2) as ptp,
            tc.tile_pool(name="psx", bufs=3, space="PSUM") as psx,
            tc.tile_pool(name="pso", bufs=2, space="PSUM") as pso,
            tc.tile_pool(name="op", bufs=1) as op,
        ):
            fold_bins = []
            for u in range(8):
                if u not in pe_units:
                    fold_bins += [(u // 4, 2 * (u % 4)),
                                  (u // 4, 2 * (u % 4) + 1)]
            oacc = op.tile([C, B_LOC, H_OUT, W_OUT], f32, tag="oacc")
            s1acc = op.tile([C, len(fold_bins), W], f32, tag="s1acc")
            s2acc = op.tile([C, len(fold_bins), W], f32, tag="s2acc")
            eps_t = op.tile([C, 1], f32, tag="eps")
            nc.vector.memset(eps_t[:], float(EPS))
            # 0/1 selector [128h, 2]: col j = 1 iff j == h // 64
            sel2f = op.tile([128, 2], f32, tag="sel2f")
            nc.vector.memset(sel2f[:], 1.0)
            nc.gpsimd.affine_select(
                out=sel2f[:], in_=sel2f[:], pattern=[[-KH, 2]],
                compare_op=Alu.is_ge, fill=0.0, base=0, channel_multiplier=1,
            )
            nc.gpsimd.affine_select(
                out=sel2f[:], in_=sel2f[:], pattern=[[KH, 2]],
                compare_op=Alu.is_ge, fill=0.0, base=KH - 1,
                channel_multiplier=-1,
            )
            sel2b = op.tile([128, 2], bf16, tag="sel2b")
            nc.vector.tensor_copy(sel2b[:], sel2f[:])
            # kw selector [128w, 16]: col j = 1 iff j == w // 8
            kwsel = op.tile([128, W_OUT], f32, tag="kwsel")
            nc.vector.memset(kwsel[:], 1.0)
            nc.gpsimd.affine_select(
                out=kwsel[:], in_=kwsel[:], pattern=[[-KW, W_OUT]],
                compare_op=Alu.is_ge, fill=0.0, base=0, channel_multiplier=1,
            )
            nc.gpsimd.affine_select(
                out=kwsel[:], in_=kwsel[:], pattern=[[KW, W_OUT]],
                compare_op=Alu.is_ge, fill=0.0, base=KW - 1,
                channel_multiplier=-1,
            )

            loop_cm = (tc.For_i(0, loop_reps, 1) if loop_reps > 1
                       else contextlib.nullcontext())
            with loop_cm:
                fi = pi = 0
                for u in range(8):
                    b, hc = u // 4, u % 4
                    if u in pe_units:
                        idx = pi
                        pi += 1
                        ps_x = psx.tile([128, 2 * C], f32, tag="ps_x")
                        ps_q = psx.tile([128, 2 * C], f32, tag="ps_q")
                        for hv in range(2):
                            xb = xbp.tile([128, 64, W], bf16, tag="xb")
                            nc.sync.dma_start(
                                out=xb[:],
                                in_=xp_in[idx, :, 64 * hv:64 * (hv + 1), :],
                            )
                            sqb = sqp.tile([128, 64, W], bf16, tag="sqb")
                            if u in sq_dve_units:
                                nc.vector.tensor_mul(sqb[:], xb[:], xb[:])
                            else:
                                nc.scalar.activation(sqb[:], xb[:], Act.Square)
                            for c0 in range(64):
                                c = 64 * hv + c0
                                nc.tensor.matmul(
                                    ps_x[:, 2 * c:2 * c + 2],
                                    xb[:, c0, :], sel2b[:],
                                    start=True, stop=True,
                                )
                                nc.tensor.matmul(
                                    ps_q[:, 2 * c:2 * c + 2],
                                    sqb[:, c0, :], sel2b[:],
                                    start=True, stop=True,
                                )
                        mean = ptp.tile([128, 2 * C], f32, tag="pmean")
                        m2 = ptp.tile([128, 2 * C], f32, tag="pm2")
                        var = ptp.tile([128, 2 * C], f32, tag="pvar")
                        pstd = ptp.tile([128, 2, C], f32, tag="pstd")
                        nc.vector.tensor_scalar_mul(mean[:], ps_x[:], 1.0 / KH)
                        nc.vector.tensor_mul(m2[:], mean[:], mean[:])
                        nc.vector.scalar_tensor_tensor(
                            out=var[:], in0=ps_q[:], scalar=1.0 / KH,
                            in1=m2[:], op0=Alu.mult, op1=Alu.subtract,
                        )
                        nc.scalar.activation(
                            pstd[:].transpose([0, 2, 1]),
                            var[:].rearrange("p (c t) -> p c t", t=2),
                            Act.Sqrt, bias=eps_t[:], scale=1.0,
                        )
                        ps_o = pso.tile([128, 2, W_OUT], f32, tag="ps_o")
                        for bin_ in range(2):
                            nc.tensor.matmul(
                                ps_o[:, bin_, :],
                                pstd[:, bin_, :], kwsel[:],
                                start=True, stop=True,
                            )
                        nc.vector.tensor_copy(
                            oacc[:, b, 2 * hc:2 * hc + 2, :], ps_o[:])
                    else:
                        idx = fi
                        fi += 1
                        for t in range(2):
                            ih = 2 * hc + t
                            k = fold_bins.index((b, ih))
                            xb = xbp.tile([C, KH, W], bf16, tag="xb")
                            nc.sync.dma_start(
                                out=xb[:], in_=xn_in[b, :, hc, t, :, :])
                            sqb = sqp.tile([C, KH, W], bf16, tag="sqb")
                            nc.scalar.activation(sqb[:], xb[:], Act.Square)
                            xl1 = glp.tile([C, KH // 2, W], bf16, tag="xl1")
                            nc.vector.tensor_add(
                                xl1[:], xb[:, 0:32, :], xb[:, 32:64, :])
                            # x chain: fresh-tile halving, bf16 deep
                            x16t = ftp.tile([C, 16, W], bf16, tag="x16")
                            x8t = ftp.tile([C, 8, W], bf16, tag="x8")
                            x4t = ftp.tile([C, 4, W], bf16, tag="x4")
                            x2t = ftp.tile([C, 2, W], bf16, tag="x2")
                            nc.vector.tensor_add(
                                x16t[:], xl1[:, 0:16, :], xl1[:, 16:32, :])
                            nc.vector.tensor_add(
                                x8t[:], x16t[:, 0:8, :], x16t[:, 8:16, :])
                            nc.vector.tensor_add(
                                x4t[:], x8t[:, 0:4, :], x8t[:, 4:8, :])
                            nc.vector.tensor_add(
                                x2t[:], x4t[:, 0:2, :], x4t[:, 2:4, :])
                            nc.vector.tensor_add(
                                s1acc[:, k, :], x2t[:, 0, :], x2t[:, 1, :])
                            # sq chain: fresh tiles, f32 from 4 rows
                            q32t = ftp.tile([C, 32, W], bf16, tag="q32")
                            q16t = ftp.tile([C, 16, W], bf16, tag="q16")
                            q8t = ftp.tile([C, 8, W], bf16, tag="q8")
                            q4t = ftp.tile([C, 4, W], f32, tag="q4")
                            q2t = ftp.tile([C, 2, W], f32, tag="q2")
                            nc.vector.tensor_add(
                                q32t[:], sqb[:, 0:32, :], sqb[:, 32:64, :])
                            nc.vector.tensor_add(
                                q16t[:], q32t[:, 0:16, :], q32t[:, 16:32, :])
                            nc.vector.tensor_add(
                                q8t[:], q16t[:, 0:8, :], q16t[:, 8:16, :])
                            nc.vector.tensor_add(
                                q4t[:], q8t[:, 0:4, :], q8t[:, 4:8, :])
                            nc.vector.tensor_add(
                                q2t[:], q4t[:, 0:2, :], q4t[:, 2:4, :])
                            nc.vector.tensor_add(
                                s2acc[:, k, :], q2t[:, 0, :], q2t[:, 1, :])
                # batched fold VS over all fold bins
                nfb = len(fold_bins)
                fmean = vbp.tile([C, nfb, W], f32, tag="fmean")
                fm2 = vbp.tile([C, nfb, W], f32, tag="fm2")
                fvar = vbp.tile([C, nfb, W], f32, tag="fvar")
                fstd = vbp.tile([C, nfb, W], f32, tag="fstd")
                nc.vector.tensor_scalar_mul(fmean[:], s1acc[:], 1.0 / KH)
                nc.vector.tensor_mul(fm2[:], fmean[:], fmean[:])
                nc.vector.scalar_tensor_tensor(
                    out=fvar[:], in0=s2acc[:], scalar=1.0 / KH,
                    in1=fm2[:], op0=Alu.mult, op1=Alu.subtract,
                )
                nc.scalar.activation(
                    fstd[:].rearrange("p k w -> p (k w)"),
                    fvar[:].rearrange("p k w -> p (k w)"),
                    Act.Sqrt, bias=eps_t[:], scale=1.0,
                )
                for k, (b, ih) in enumerate(fold_bins):
                    nc.vector.reduce_sum(
                        out=oacc[:, b, ih, :],
                        in_=fstd[:, k, :].rearrange("p (g k) -> p g k", k=KW),
                        axis=mybir.AxisListType.X,
                    )
            nc.sync.dma_start(out=out.transpose([1, 0, 2, 3]), in_=oacc[:])
    nc.finalize()
    return nc


def kernel(x):
    import ml_dtypes
    from concourse.bass_utils import run_bass_kernel_spmd

    x = np.ascontiguousarray(np.asarray(x, dtype=np.float32))
    assert x.shape == (B, C, H, W), x.shape
    x16 = x.astype(ml_dtypes.bfloat16)

    if "nc" not in _CACHE:
        _CACHE["nc"] = build()
    nc = _CACHE["nc"]

    in_maps = [
        make_inputs(x16[i * B_LOC:(i + 1) * B_LOC])
        for i in range(N_CORES)
    ]
    last_err = None
    for _ in range(3):
        try:
            res = run_bass_kernel_spmd(nc, in_maps, core_ids=list(range(N_CORES)))
            break
        except Exception as e:  # transient axon/device hiccups
            last_err = e
    else:
        raise last_err
    return np.concatenate([np.asarray(r["out"]) for r in res.results], axis=0)
